# revision 1
# baseline (speedup 1.0000x reference)
"""Cross-attention kernel for Trainium2 (Bass/Tile), data-parallel over batch on 8 cores.

Reference computation (per batch sample b):
    Q = text @ Wq.T + bq          [T, D]
    K = features @ Wk.T + bk      [P, D]
    scores = Q @ K.T / sqrt(D)    [T, P]
    attn = softmax(scores, -1)
    out = attn @ features         [T, D]

Per-core schedule (one batch sample per NeuronCore):
    Phase A: KT[d,p] = sum_x WkT[x,d]*featT[x,p] + bk  -> kt_dram     (d on partitions)
    Phase B: QT[d,t] = sum_x WqT[x,d]*textT[x,t] + bq  -> qt_dram
    Phase C: software-pipelined per 128-row t-tile:
        scores[t,p] = sum_d QT[d,t]*KT[d,p]   (PSUM, 2 halves of 288)
        softmax over the free dim (max via DVE, exp via ACT with fused 1/sqrt(D)
        scale; normalization deferred to the output eviction); scores of the next
        t-tile are emitted before this tile's transposes so the PE never waits
        on the softmax latency.
        attnT via PE transpose
        out[t,d] = sum_p attnT[p,t]*features[p,d], scaled by 1/rowsum on eviction

All matmuls use float32r (fp32 storage, full PE rate for free-dim >= 256).
Large SBUF residents are loaded as per-chunk tiles so allocation (and hence the
DMA) starts incrementally as the previous phase's space frees up.
"""

import numpy as np

import concourse.bacc as bacc
import concourse.mybir as mybir
import concourse.tile as tile
from concourse.bass_utils import run_bass_kernel_spmd
from concourse.masks import make_identity

F32 = mybir.dt.float32
F32R = mybir.dt.float32r

# Full problem dims (hardcoded per harness contract)
T_FULL, P_FULL, D_FULL, X_FULL = 2048, 576, 4096, 4096
N_CORES = 8


def build_attention_nc(T=T_FULL, P=P_FULL, D=D_FULL, X=X_FULL, repeat=1):
    assert T % 128 == 0 and D % 128 == 0 and X % 128 == 0
    XO, DT, TT = X // 128, D // 128, T // 128
    PC = -(-P // 128)              # p-chunks for the attended contraction
    P_LAST = P - (PC - 1) * 128
    SCH = P // 2                   # scores half width (288 for P=576); >=256 keeps f32r fast
    assert P % 2 == 0 and SCH <= 512
    TCB = min(1024, T)             # phase-B resident textT chunk
    NTCB = T // TCB
    NB = min(512, TCB)             # phase-B psum free width
    DC = min(512, D)               # attended d chunk
    NDC = D // DC
    scale = 1.0 / float(np.sqrt(D))

    nc = bacc.Bacc()
    textT = nc.dram_tensor("textT", [X, T], F32R, kind="ExternalInput")
    featT = nc.dram_tensor("featT", [X, P], F32R, kind="ExternalInput")
    feat = nc.dram_tensor("feat", [P, D], F32R, kind="ExternalInput")
    wq = nc.dram_tensor("wq", [DT, 128, XO, 128], F32R, kind="ExternalInput")
    wk = nc.dram_tensor("wk", [DT, 128, XO, 128], F32R, kind="ExternalInput")
    bq = nc.dram_tensor("bq", [128, DT], F32, kind="ExternalInput")
    bk = nc.dram_tensor("bk", [128, DT], F32, kind="ExternalInput")
    out = nc.dram_tensor("out", [T, D], F32, kind="ExternalOutput")
    qt_d = nc.dram_tensor("qt_tmp", [DT, 128, T], F32R)
    kt_d = nc.dram_tensor("kt_tmp", [DT, 128, P], F32R)

    textT_v = textT.rearrange("(xo p) t -> p xo t", p=128)
    featT_v = featT.rearrange("(xo p) q -> p xo q", p=128)
    out_v = out.rearrange("(tt p) d -> p tt d", p=128)

    AX = mybir.AxisListType.X
    ALU = mybir.AluOpType
    EXP = mybir.ActivationFunctionType.Exp

    with tile.TileContext(nc) as tc:
        with (
            tc.tile_pool(name="psum", bufs=8, space="PSUM") as psum,
            tc.tile_pool(name="const", bufs=1) as const,
        ):
            ident = const.tile([128, 128], F32)
            make_identity(nc, ident[:])
            bq_sb = const.tile([128, DT], F32, tag="bq")
            nc.sync.dma_start(bq_sb[:], bq[:])
            bk_sb = const.tile([128, DT], F32, tag="bk")
            nc.sync.dma_start(bk_sb[:], bk[:])

            def _emit_body():
              from contextlib import ExitStack
              es_a, es_b, es_kt1, es_c = ExitStack(), ExitStack(), ExitStack(), ExitStack()
              with es_a, es_b, es_kt1, es_c:
                  a_w = es_a.enter_context(tc.tile_pool(name="a_w", bufs=3, side="right"))
                  a_rhs = es_a.enter_context(tc.tile_pool(name="a_rhs", bufs=1, side="right"))
                  a_out = es_a.enter_context(tc.tile_pool(name="a_out", bufs=3, side="right"))
                  b_w = es_b.enter_context(tc.tile_pool(name="b_w", bufs=2))
                  b_out = es_b.enter_context(tc.tile_pool(name="b_out", bufs=3))
                  b_rhs1 = es_b.enter_context(tc.tile_pool(name="b_rhs1", bufs=1))
                  TT_EARLY = min(9, XO)   # textT piece tags allocated alongside phase A

                  # ---------- Phase A: KT -> kt_dram ----------
                  # first Wk tile ahead of the featT pieces so the PE can start early
                  wk_t = {0: a_w.tile([128, XO, 128], F32R, tag="aw", name="wk_sb")}
                  nc.sync.dma_start(wk_t[0][:], wk[0])
                  ft_t = []
                  for xo in range(XO):
                      t_ = a_rhs.tile([128, P], F32R, tag=f"ft{xo}")
                      nc.sync.dma_start(t_[:], featT_v[:, xo, :])
                      ft_t.append(t_)

                  tt_t = {}

                  def load_tt_piece(tcb, xo):
                      pool = b_rhs1 if xo < TT_EARLY else b_rhs2
                      t_ = pool.tile([128, TCB], F32R, tag=f"tt{xo}", name="tt_sb")
                      nc.sync.dma_start(
                          t_[:], textT_v[:, xo, tcb * TCB:(tcb + 1) * TCB])
                      tt_t[(tcb, xo)] = t_

                  for dt in range(DT):
                      if dt not in wk_t:
                          wk_t[dt] = a_w.tile([128, XO, 128], F32R, tag="aw", name="wk_sb")
                          nc.sync.dma_start(wk_t[dt][:], wk[dt])
                      w_sb = wk_t[dt]
                      ps0 = psum.tile([128, 512], F32, tag="ps")
                      ps1 = psum.tile([128, 512], F32, tag="ps")
                      for xo in range(XO):
                          nc.tensor.matmul(
                              ps0[:, :SCH], w_sb[:, xo, :], ft_t[xo][:, 0:SCH],
                              start=(xo == 0), stop=(xo == XO - 1),
                          )
                          nc.tensor.matmul(
                              ps1[:, :SCH], w_sb[:, xo, :], ft_t[xo][:, SCH:2 * SCH],
                              start=(xo == 0), stop=(xo == XO - 1),
                          )
                      o = a_out.tile([128, P], F32R, tag="ao")
                      nc.vector.tensor_scalar_add(o[:, 0:SCH], ps0[:, :SCH], bk_sb[:, dt:dt + 1])
                      nc.vector.tensor_scalar_add(o[:, SCH:2 * SCH], ps1[:, :SCH], bk_sb[:, dt:dt + 1])
                      nc.sync.dma_start(kt_d[dt], o[:])
                      if dt < TT_EARLY:
                          load_tt_piece(0, dt)

                  es_a.close()
                  b_rhs2 = es_b.enter_context(tc.tile_pool(name="b_rhs2", bufs=1))
                  c_kt1 = es_kt1.enter_context(tc.tile_pool(name="c_kt1", bufs=1, side="right"))
                  KT_EARLY = min(12, DT)  # KT piece tags prefetched during phase B
                  for xo in range(TT_EARLY, XO):
                      load_tt_piece(0, xo)

                  # ---------- Phase B: QT -> qt_dram ----------
                  def emit_b_pass(tcb):
                      for dt in range(DT):
                          w_sb = b_w.tile([128, XO, 128], F32R, tag="bw", name="wq_sb")
                          nc.sync.dma_start(w_sb[:], wq[dt])
                          for nb in range(TCB // NB):
                              ps = psum.tile([128, 512], F32, tag="ps")
                              for xo in range(XO):
                                  nc.tensor.matmul(
                                      ps[:, :NB], w_sb[:, xo, :],
                                      tt_t[(tcb, xo)][:, nb * NB:(nb + 1) * NB],
                                      start=(xo == 0), stop=(xo == XO - 1),
                                  )
                              o = b_out.tile([128, NB], F32R, tag="bo")
                              nc.vector.tensor_scalar_add(o[:], ps[:, :NB], bq_sb[:, dt:dt + 1])
                              t0 = tcb * TCB + nb * NB
                              nc.sync.dma_start(qt_d[dt, :, t0:t0 + NB], o[:])

                  emit_b_pass(0)
                  # phase-C KT pieces: data-ready (phase A done); loaded in B's DMA slack
                  kt_t = []
                  for dt in range(KT_EARLY):
                      t_ = c_kt1.tile([128, P], F32R, tag=f"kt{dt}", name="kt_sb")
                      nc.sync.dma_start(t_[:], kt_d[dt])
                      kt_t.append(t_)
                  for tcb in range(1, NTCB):
                      for xo in range(XO):
                          load_tt_piece(tcb, xo)
                      emit_b_pass(tcb)

                  es_b.close()
                  c_kt2 = es_c.enter_context(tc.tile_pool(name="c_kt2", bufs=1))
                  c_feat = es_c.enter_context(tc.tile_pool(name="c_feat", bufs=1))
                  c_qt = es_c.enter_context(tc.tile_pool(name="c_qt", bufs=2))
                  c_attn = es_c.enter_context(tc.tile_pool(name="c_attn", bufs=2))
                  c_attnT = es_c.enter_context(tc.tile_pool(name="c_attnT", bufs=2))
                  c_stat = es_c.enter_context(tc.tile_pool(name="c_stat", bufs=4))
                  c_out = es_c.enter_context(tc.tile_pool(name="c_out", bufs=3))
                  for dt in range(KT_EARLY, DT):
                      t_ = c_kt2.tile([128, P], F32R, tag=f"kt{dt}", name="kt_sb")
                      nc.sync.dma_start(t_[:], kt_d[dt])
                      kt_t.append(t_)

                  # ---------- Phase C: scores/softmax/attended (software-pipelined) ----------
                  def emit_scores(tt):
                      qt_sb = c_qt.tile([128, DT, 128], F32R, tag="cqt")
                      nc.sync.dma_start(
                          qt_sb[:],
                          qt_d[:, :, tt * 128:(tt + 1) * 128].rearrange("dt p t -> p dt t"),
                      )
                      ps0 = psum.tile([128, 512], F32, tag="ps")
                      ps1 = psum.tile([128, 512], F32, tag="ps")
                      for dt in range(DT):
                          nc.tensor.matmul(
                              ps0[:, :SCH], qt_sb[:, dt, :], kt_t[dt][:, 0:SCH],
                              start=(dt == 0), stop=(dt == DT - 1),
                          )
                          nc.tensor.matmul(
                              ps1[:, :SCH], qt_sb[:, dt, :], kt_t[dt][:, SCH:2 * SCH],
                              start=(dt == 0), stop=(dt == DT - 1),
                          )
                      return ps0, ps1

                  cur = emit_scores(0)

                  feat_t = []
                  for pc in range(PC):
                      rows = 128 if pc < PC - 1 else P_LAST
                      t_ = c_feat.tile([128, D], F32R, tag=f"feat{pc}")
                      nc.sync.dma_start(t_[:rows, :], feat[pc * 128:pc * 128 + rows, :])
                      feat_t.append(t_)

                  for tt in range(TT):
                      ps0, ps1 = cur
                      # softmax stats (DVE/ACT) - normalization deferred to eviction
                      mx0 = c_stat.tile([128, 1], F32, tag="mx0")
                      mx1 = c_stat.tile([128, 1], F32, tag="mx1")
                      nc.vector.tensor_reduce(mx0[:], ps0[:, :SCH], AX, ALU.max)
                      nc.vector.tensor_reduce(mx1[:], ps1[:, :SCH], AX, ALU.max)
                      negmax = c_stat.tile([128, 1], F32, tag="negmax")
                      nc.vector.tensor_tensor(negmax[:], mx0[:], mx1[:], ALU.max)
                      nc.vector.tensor_scalar_mul(negmax[:], negmax[:], -scale)
                      attn = c_attn.tile([128, P], F32, tag="attn")
                      nc.scalar.activation(attn[:, 0:SCH], ps0[:, :SCH], EXP, bias=negmax[:], scale=scale)
                      nc.scalar.activation(attn[:, SCH:2 * SCH], ps1[:, :SCH], EXP, bias=negmax[:], scale=scale)
                      ssum = c_stat.tile([128, 1], F32, tag="ssum")
                      nc.vector.tensor_reduce(ssum[:], attn[:], AX, ALU.add)
                      rsum = c_stat.tile([128, 1], F32, tag="rsum")
                      nc.vector.reciprocal(rsum[:], ssum[:])

                      # pipeline: next tile's scores keep the PE busy during softmax
                      if tt + 1 < TT:
                          cur = emit_scores(tt + 1)

                      # transpose attn -> attnT
                      atT = c_attnT.tile([128, PC, 128], F32, tag="atT")
                      for pc in range(PC):
                          cols = 128 if pc < PC - 1 else P_LAST
                          pst = psum.tile([128, 512], F32, tag="ps")
                          nc.tensor.transpose(pst[:cols, :128], attn[:, pc * 128:pc * 128 + cols], ident[:])
                          nc.vector.tensor_copy(atT[:cols, pc, :].bitcast(F32R), pst[:cols, :128])
                      # attended
                      for dc in range(NDC):
                          pa = psum.tile([128, 512], F32, tag="ps")
                          for pc in range(PC):
                              rows = 128 if pc < PC - 1 else P_LAST
                              nc.tensor.matmul(
                                  pa[:, :DC], atT[:rows, pc, :].bitcast(F32R),
                                  feat_t[pc][:rows, dc * DC:(dc + 1) * DC],
                                  start=(pc == 0), stop=(pc == PC - 1),
                              )
                          o = c_out.tile([128, DC], F32, tag="co")
                          nc.vector.tensor_scalar_mul(o[:], pa[:, :DC], rsum[:])
                          nc.sync.dma_start(out_v[:, tt, dc * DC:(dc + 1) * DC], o[:])

            if repeat > 1:
                with tc.For_i(0, repeat, 1):
                    _emit_body()
            else:
                _emit_body()

    nc.compile()
    return nc


def prep_core_inputs(text_i, feat_i, wq_pre, wk_pre, bq_r, bk_r):
    return {
        "textT": np.ascontiguousarray(text_i.T),
        "featT": np.ascontiguousarray(feat_i.T),
        "feat": np.ascontiguousarray(feat_i),
        "wq": wq_pre,
        "wk": wk_pre,
        "bq": bq_r,
        "bk": bk_r,
    }


def prep_weights(Wq, bq, Wk, bk, D=None, X=None):
    D = D or Wq.shape[0]
    X = X or Wq.shape[1]
    DT, XO = D // 128, X // 128
    # w_pre[dt, p, xo, d] = W[dt*128+d, xo*128+p]
    wq_pre = np.ascontiguousarray(
        np.asarray(Wq, np.float32).reshape(DT, 128, XO, 128).transpose(0, 3, 2, 1))
    wk_pre = np.ascontiguousarray(
        np.asarray(Wk, np.float32).reshape(DT, 128, XO, 128).transpose(0, 3, 2, 1))
    bq_r = np.ascontiguousarray(np.asarray(bq, np.float32).reshape(DT, 128).T)
    bk_r = np.ascontiguousarray(np.asarray(bk, np.float32).reshape(DT, 128).T)
    return wq_pre, wk_pre, bq_r, bk_r


_NC_CACHE = {}


def kernel(text, features, Wq, bq, Wk, bk):
    text = np.asarray(text, np.float32)
    features = np.asarray(features, np.float32)
    B, T, X = text.shape
    _, P, _ = features.shape
    D = Wq.shape[0]
    key = (T, P, D, X)
    if key not in _NC_CACHE:
        _NC_CACHE[key] = build_attention_nc(T, P, D, X)
    nc = _NC_CACHE[key]

    wq_pre, wk_pre, bq_r, bk_r = prep_weights(Wq, bq, Wk, bk, D, X)
    in_maps = [
        prep_core_inputs(text[i], features[i], wq_pre, wk_pre, bq_r, bk_r)
        for i in range(B)
    ]
    res = run_bass_kernel_spmd(nc, in_maps, list(range(B)))
    return np.stack([res.results[i]["out"] for i in range(B)], axis=0)



# revision 8
# speedup vs baseline: 2.1226x; 2.1226x over previous
"""Cross-attention kernel for Trainium2 (Bass/Tile), data-parallel over batch on 8 cores.

Reference computation (per batch sample b):
    Q = text @ Wq.T + bq          [T, D]
    K = features @ Wk.T + bk      [P, D]
    scores = Q @ K.T / sqrt(D)    [T, P]
    attn = softmax(scores, -1)
    out = attn @ features         [T, D]

The dominant cost in this deployment is host->device shipping of the call
arguments (~11.8 GB/s, serialized), so everything is shipped once, in bf16,
with no redundancy:
  - text is shipped pre-transposed (textT, bf16, 16 MB/core)
  - features shipped once (bf16, 4.5 MB/core); featT is built on device via
    PE transposes
  - Wq/Wk are shipped as 1/8 column shards of W.T (bf16, 4 MB each/core) and
    AllGathered across the 8 cores over NeuronLink at kernel start
  - the output is written in bf16 and widened to f32 on the host

Per-core schedule (one batch sample per NeuronCore):
    AG(wkT), AG(wqT)                      (gpsimd, overlaps the next steps)
    feat -> SBUF; featT via PE transpose
    Phase A: KT[d,p] = sum_x WkT[x,d]*featT[x,p] + bk   (SBUF-resident, bf16)
    8 rounds over T (TCB=256 rows each), software-pipelined:
      Phase B: QT[d,t] = sum_x WqT[x,d]*textT[x,t] + bq  (SBUF-resident, bf16)
      Phase C: per 128-row t-tile: scores (PSUM, 2x288), softmax (max via DVE,
        exp via ACT with fused 1/sqrt(D) scale, normalization deferred to the
        output eviction), attnT via PE transpose, out = attnT.T @ feat.
All matmul operands are bf16 (full PE rate); accumulation is fp32 in PSUM.
"""

import numpy as np
import ml_dtypes

import concourse.bacc as bacc
import concourse.mybir as mybir
import concourse.tile as tile
from concourse.bass_utils import run_bass_kernel_spmd
from concourse.masks import make_identity

F32 = mybir.dt.float32
BF16 = mybir.dt.bfloat16
BF16_NP = ml_dtypes.bfloat16

# Full problem dims (hardcoded per harness contract)
T_FULL, P_FULL, D_FULL, X_FULL = 2048, 576, 4096, 4096
N_CORES = 8
DSH = D_FULL // N_CORES  # weight-shard width shipped to each core


def build_attention_nc(T=T_FULL, P=P_FULL, D=D_FULL, X=X_FULL, n_cores=N_CORES,
                       repeat=1):
    assert T % 128 == 0 and D % 128 == 0 and X % 128 == 0
    XO, DT, TT = X // 128, D // 128, T // 128
    DSHL = D // n_cores             # weight shard width
    JD = DSHL // 128                # d-tiles per shard
    PC = -(-P // 128)               # p-chunks for the attended contraction
    P_LAST = P - (PC - 1) * 128
    SCH = P // 2                    # scores half width (288 for P=576)
    assert P % 2 == 0 and SCH <= 512
    TCB = 256                       # rounds granularity over T
    NR = T // TCB                   # number of B/C rounds
    TPR = TCB // 128                # t-tiles per round
    DC = min(512, D)                # attended d chunk
    NDC = D // DC
    scale = 1.0 / float(np.sqrt(D))

    nc = bacc.Bacc()
    textT = nc.dram_tensor("textT", [X, T], BF16, kind="ExternalInput")
    feat = nc.dram_tensor("feat", [P, D], BF16, kind="ExternalInput")
    wq_sh = nc.dram_tensor("wq_sh", [X, DSHL], BF16, kind="ExternalInput")
    wk_sh = nc.dram_tensor("wk_sh", [X, DSHL], BF16, kind="ExternalInput")
    bq = nc.dram_tensor("bq", [128, DT], F32, kind="ExternalInput")
    bk = nc.dram_tensor("bk", [128, DT], F32, kind="ExternalInput")
    out = nc.dram_tensor("out", [T, D], BF16, kind="ExternalOutput")

    textT_v = textT.rearrange("(xo p) t -> p xo t", p=128)
    out_v = out.rearrange("(tt p) d -> p tt d", p=128)

    AX = mybir.AxisListType.X
    ALU = mybir.AluOpType
    EXP = mybir.ActivationFunctionType.Exp

    with tile.TileContext(nc) as tc:
        with (
            tc.tile_pool(name="psum", bufs=8, space="PSUM") as psum,
            tc.tile_pool(name="const", bufs=1) as const,
            tc.tile_pool(name="dram", bufs=1, space="DRAM") as dram,
        ):
            # --- weight all-gather over NeuronLink (gpsimd queue) ---
            wk_in = dram.tile([X, DSHL], BF16, tag="wk_in")
            wk_full = dram.tile([n_cores * X, DSHL], BF16, tag="wk_full")
            wq_in = dram.tile([X, DSHL], BF16, tag="wq_in")
            wq_full = dram.tile([n_cores * X, DSHL], BF16, tag="wq_full")
            nc.gpsimd.dma_start(wk_in[:], wk_sh[:])
            nc.gpsimd.collective_compute(
                "AllGather", ALU.bypass,
                replica_groups=[list(range(n_cores))],
                ins=[wk_in.opt()], outs=[wk_full.opt()],
            )
            nc.gpsimd.dma_start(wq_in[:], wq_sh[:])
            nc.gpsimd.collective_compute(
                "AllGather", ALU.bypass,
                replica_groups=[list(range(n_cores))],
                ins=[wq_in.opt()], outs=[wq_full.opt()],
            )

            ident_f = const.tile([128, 128], F32, tag="idf")
            make_identity(nc, ident_f[:])
            ident_b = const.tile([128, 128], BF16, tag="idb")
            make_identity(nc, ident_b[:])
            bq_sb = const.tile([128, DT], F32, tag="bq")
            nc.sync.dma_start(bq_sb[:], bq[:])
            bk_sb = const.tile([128, DT], F32, tag="bk")
            nc.sync.dma_start(bk_sb[:], bk[:])

            def _emit_body():
              from contextlib import ExitStack
              es_res, es_a = ExitStack(), ExitStack()
              with es_res, es_a:
                # --- residents: feat + KT live for the whole body ---
                feat_pool = es_res.enter_context(tc.tile_pool(name="feat", bufs=1))
                kt_pool = es_res.enter_context(tc.tile_pool(name="kt", bufs=1))
                # --- phase-A-only: featT + wk stationaries ---
                featT_pool = es_a.enter_context(
                    tc.tile_pool(name="featT", bufs=1, side="right"))
                a_w = es_a.enter_context(
                    tc.tile_pool(name="a_w", bufs=2, side="right"))

                feat_t = []
                for pc in range(PC):
                    rows = 128 if pc < PC - 1 else P_LAST
                    t_ = feat_pool.tile([128, D], BF16, tag=f"feat{pc}")
                    nc.sync.dma_start(t_[:rows, :], feat[pc * 128:pc * 128 + rows, :])
                    feat_t.append(t_)

                # featT[x, p] via PE transpose of feat tiles
                featT_t = []
                for xo in range(XO):
                    ft = featT_pool.tile([128, P], BF16, tag=f"ft{xo}")
                    featT_t.append(ft)
                for xo in range(XO):
                    for pc in range(PC):
                        rows = 128 if pc < PC - 1 else P_LAST
                        pst = psum.tile([128, 1024], BF16, tag="ps")
                        nc.tensor.transpose(
                            pst[:128, :rows],
                            feat_t[pc][:rows, xo * 128:(xo + 1) * 128],
                            ident_b[:rows, :rows])
                        nc.vector.tensor_copy(
                            featT_t[xo][:, pc * 128:pc * 128 + rows],
                            pst[:128, :rows])

                # ---------- Phase A: KT (SBUF-resident bf16) ----------
                kt_t = []
                for dt in range(DT):
                    kt_t.append(kt_pool.tile([128, P], BF16, tag=f"kt{dt}",
                                             name="kt_sb"))
                wk_fv = wk_full[:]
                for c8 in range(n_cores):
                    w_sb = a_w.tile([128, XO, DSHL], BF16, tag="aw")
                    nc.sync.dma_start(
                        w_sb[:],
                        wk_fv[c8 * X:(c8 + 1) * X, :].rearrange(
                            "(xo p) d -> p xo d", p=128))
                    for j in range(JD):
                        dt = c8 * JD + j
                        ps0 = psum.tile([128, 512], F32, tag="ps")
                        ps1 = psum.tile([128, 512], F32, tag="ps")
                        for xo in range(XO):
                            nc.tensor.matmul(
                                ps0[:, :SCH], w_sb[:, xo, j * 128:(j + 1) * 128],
                                featT_t[xo][:, 0:SCH],
                                start=(xo == 0), stop=(xo == XO - 1))
                            nc.tensor.matmul(
                                ps1[:, :SCH], w_sb[:, xo, j * 128:(j + 1) * 128],
                                featT_t[xo][:, SCH:2 * SCH],
                                start=(xo == 0), stop=(xo == XO - 1))
                        nc.vector.tensor_scalar_add(
                            kt_t[dt][:, 0:SCH], ps0[:, :SCH], bk_sb[:, dt:dt + 1])
                        nc.vector.tensor_scalar_add(
                            kt_t[dt][:, SCH:2 * SCH], ps1[:, :SCH], bk_sb[:, dt:dt + 1])

                es_a.close()  # frees featT + wk tiles for the B/C rounds

                es_bc = ExitStack()
                with es_bc:
                    b_w = es_bc.enter_context(
                        tc.tile_pool(name="b_w", bufs=2, side="right"))
                    b_chunk = es_bc.enter_context(
                        tc.tile_pool(name="b_chunk", bufs=1, side="right"))
                    b_qt = es_bc.enter_context(tc.tile_pool(name="b_qt", bufs=1))
                    c_attn = es_bc.enter_context(tc.tile_pool(name="c_attn", bufs=2))
                    c_attnT = es_bc.enter_context(tc.tile_pool(name="c_attnT", bufs=2))
                    c_stat = es_bc.enter_context(tc.tile_pool(name="c_stat", bufs=8))
                    c_out = es_bc.enter_context(tc.tile_pool(name="c_out", bufs=3))

                    CHW = 2 * TCB  # textT chunk width (one load per 2 rounds)

                    def emit_scores(qt_sb, ti):
                        ps0 = psum.tile([128, 512], F32, tag="ps")
                        ps1 = psum.tile([128, 512], F32, tag="ps")
                        for dt in range(DT):
                            nc.tensor.matmul(
                                ps0[:, :SCH], qt_sb[:, dt, ti * 128:(ti + 1) * 128],
                                kt_t[dt][:, 0:SCH],
                                start=(dt == 0), stop=(dt == DT - 1))
                            nc.tensor.matmul(
                                ps1[:, :SCH], qt_sb[:, dt, ti * 128:(ti + 1) * 128],
                                kt_t[dt][:, SCH:2 * SCH],
                                start=(dt == 0), stop=(dt == DT - 1))
                        return ps0, ps1

                    chunk = None
                    for q in range(NR):
                        # ---------- Phase B: QT for this round ----------
                        if q % 2 == 0:
                            chunk = b_chunk.tile([128, XO, CHW], BF16, tag="ch")
                            nc.sync.dma_start(
                                chunk[:], textT_v[:, :, q * TCB:q * TCB + CHW])
                        coff = (q % 2) * TCB
                        qt_sb = b_qt.tile([128, DT, TCB], BF16, tag="qt")
                        for c8 in range(n_cores):
                            w_sb = b_w.tile([128, XO, DSHL], BF16, tag="bw")
                            nc.sync.dma_start(
                                w_sb[:],
                                wq_full[:][c8 * X:(c8 + 1) * X, :].rearrange(
                                    "(xo p) d -> p xo d", p=128))
                            for j in range(JD):
                                dt = c8 * JD + j
                                ps = psum.tile([128, 512], F32, tag="ps")
                                for xo in range(XO):
                                    nc.tensor.matmul(
                                        ps[:, :TCB],
                                        w_sb[:, xo, j * 128:(j + 1) * 128],
                                        chunk[:, xo, coff:coff + TCB],
                                        start=(xo == 0), stop=(xo == XO - 1))
                                nc.vector.tensor_scalar_add(
                                    qt_sb[:, dt, :], ps[:, :TCB], bq_sb[:, dt:dt + 1])

                        # ---------- Phase C: 2 t-tiles of this round ----------
                        cur = emit_scores(qt_sb, 0)
                        for ti in range(TPR):
                            tt = q * TPR + ti
                            ps0, ps1 = cur
                            mx0 = c_stat.tile([128, 1], F32, tag="mx0")
                            mx1 = c_stat.tile([128, 1], F32, tag="mx1")
                            nc.vector.tensor_reduce(mx0[:], ps0[:, :SCH], AX, ALU.max)
                            nc.vector.tensor_reduce(mx1[:], ps1[:, :SCH], AX, ALU.max)
                            negmax = c_stat.tile([128, 1], F32, tag="negmax")
                            nc.vector.tensor_tensor(negmax[:], mx0[:], mx1[:], ALU.max)
                            nc.vector.tensor_scalar_mul(negmax[:], negmax[:], -scale)
                            attn = c_attn.tile([128, P], F32, tag="attn")
                            nc.scalar.activation(attn[:, 0:SCH], ps0[:, :SCH], EXP,
                                                 bias=negmax[:], scale=scale)
                            nc.scalar.activation(attn[:, SCH:2 * SCH], ps1[:, :SCH],
                                                 EXP, bias=negmax[:], scale=scale)
                            ssum = c_stat.tile([128, 1], F32, tag="ssum")
                            nc.vector.tensor_reduce(ssum[:], attn[:], AX, ALU.add)
                            rsum = c_stat.tile([128, 1], F32, tag="rsum")
                            nc.vector.reciprocal(rsum[:], ssum[:])

                            # keep the PE busy during the softmax latency
                            if ti + 1 < TPR:
                                cur = emit_scores(qt_sb, ti + 1)

                            atT = c_attnT.tile([128, PC, 128], BF16, tag="atT")
                            for pc in range(PC):
                                cols = 128 if pc < PC - 1 else P_LAST
                                pst = psum.tile([128, 512], F32, tag="ps")
                                nc.tensor.transpose(
                                    pst[:cols, :128],
                                    attn[:, pc * 128:pc * 128 + cols], ident_f[:])
                                nc.vector.tensor_copy(atT[:cols, pc, :],
                                                      pst[:cols, :128])
                            for dc in range(NDC):
                                pa = psum.tile([128, 512], F32, tag="ps")
                                for pc in range(PC):
                                    rows = 128 if pc < PC - 1 else P_LAST
                                    nc.tensor.matmul(
                                        pa[:, :DC], atT[:rows, pc, :],
                                        feat_t[pc][:rows, dc * DC:(dc + 1) * DC],
                                        start=(pc == 0), stop=(pc == PC - 1))
                                o = c_out.tile([128, DC], BF16, tag="co")
                                nc.vector.tensor_scalar_mul(o[:], pa[:, :DC], rsum[:])
                                nc.sync.dma_start(
                                    out_v[:, tt, dc * DC:(dc + 1) * DC], o[:])

            if repeat > 1:
                with tc.For_i(0, repeat, 1):
                    _emit_body()
            else:
                _emit_body()

    nc.compile()
    return nc


def prep_weights(Wq, bq, Wk, bk, D=None, X=None):
    D = D or Wq.shape[0]
    X = X or Wq.shape[1]
    DT = D // 128
    dsh = D // N_CORES
    WqT = np.asarray(Wq, np.float32).T.astype(BF16_NP)  # [X, D]
    WkT = np.asarray(Wk, np.float32).T.astype(BF16_NP)
    wq_shards = [np.ascontiguousarray(WqT[:, c * dsh:(c + 1) * dsh])
                 for c in range(N_CORES)]
    wk_shards = [np.ascontiguousarray(WkT[:, c * dsh:(c + 1) * dsh])
                 for c in range(N_CORES)]
    bq_r = np.ascontiguousarray(np.asarray(bq, np.float32).reshape(DT, 128).T)
    bk_r = np.ascontiguousarray(np.asarray(bk, np.float32).reshape(DT, 128).T)
    return wq_shards, wk_shards, bq_r, bk_r


def prep_core_inputs(text_i, feat_i, wq_sh_i, wk_sh_i, bq_r, bk_r):
    return {
        "textT": np.ascontiguousarray(np.asarray(text_i, np.float32).T).astype(BF16_NP),
        "feat": np.asarray(feat_i, np.float32).astype(BF16_NP),
        "wq_sh": wq_sh_i,
        "wk_sh": wk_sh_i,
        "bq": bq_r,
        "bk": bk_r,
    }


_NC_CACHE = {}


def kernel(text, features, Wq, bq, Wk, bk):
    text = np.asarray(text, np.float32)
    features = np.asarray(features, np.float32)
    B, T, X = text.shape
    _, P, _ = features.shape
    D = Wq.shape[0]
    key = (T, P, D, X)
    if key not in _NC_CACHE:
        _NC_CACHE[key] = build_attention_nc(T, P, D, X)
    nc = _NC_CACHE[key]

    wq_shards, wk_shards, bq_r, bk_r = prep_weights(Wq, bq, Wk, bk, D, X)
    in_maps = [
        prep_core_inputs(text[i], features[i], wq_shards[i], wk_shards[i],
                         bq_r, bk_r)
        for i in range(B)
    ]
    res = run_bass_kernel_spmd(nc, in_maps, list(range(B)))
    return np.stack(
        [np.asarray(res.results[i]["out"], np.float32) for i in range(B)], axis=0)
